# revision 1
# baseline (speedup 1.0000x reference)
"""Trainium2 Bass kernel for the Clements photonic mesh (N=512, L=512).

Column-sharded across 8 NeuronCores: every PC/MMI layer is a row operation
(left multiply), so each core evolves its own 64-column block of the
512x512 complex transfer matrix with zero communication.

Device layout (per core):
  Rows are split by parity into E (even rows 0,2,..,510) and O (odd rows),
  each linearized to 256 entries stored as two 128-partition tiles (t=0/1).
  State tiles are [128, 2(X/Y), 2(t), 64cols] fp32 (X=Re, Y=Im).
  Per-pair layer coefficients live at one partition per pair, so every
  elementwise op is a tensor_scalar / scalar_tensor_tensor with a [128,1]
  per-partition scalar slice.
  Even layers pair E[k] with O[k] (partition aligned).  Odd layers pair
  O[k] with E[k+1]; the +1 partition shift is done on the TensorEngine with
  constant shift matrices (engines cannot read cross-partition).
"""

import sys

sys.path.insert(0, "/opt/trn_rl_repo")

from contextlib import ExitStack

import numpy as np

import concourse.bass as bass
import concourse.tile as tile
from concourse import bacc, mybir
from concourse.bass_utils import run_bass_kernel_spmd

F32 = mybir.dt.float32
ALU = mybir.AluOpType
ACTF = mybir.ActivationFunctionType

N = 512
L = 512
NPAIR_E = 256
NPAIR_O = 255
TWO_PI = 6.283185307179586
HALF_PI = 1.5707963267948966
NCORES = 8
CPD = N // NCORES  # columns per device = 64

_CACHE = {}


def _build_program(n_steps=L // 2):
    """One scan step = 2 even layers + 2 odd layers (matches reference)."""
    nc = bacc.Bacc("TRN2", target_bir_lowering=False)

    par = {}
    for nm in (
        "the0", "the1", "le0", "le1", "ie0", "ie1",
        "tho0", "tho1", "lo0", "lo1", "io0", "io1",
    ):
        par[nm] = nc.declare_dram_parameter(nm, [128, L], F32, isOutput=False)
    par["pout"] = nc.declare_dram_parameter("pout", [128, 4], F32, isOutput=False)
    par["init_e"] = nc.declare_dram_parameter("init_e", [128, 2, CPD], F32, isOutput=False)
    par["init_o"] = nc.declare_dram_parameter("init_o", [128, 2, CPD], F32, isOutput=False)
    par["shifts"] = nc.declare_dram_parameter("shifts", [128, 5, 128], F32, isOutput=False)
    outv = nc.declare_dram_parameter("outv", [128, 2, 2, 2, CPD], F32, isOutput=True)

    with tile.TileContext(nc) as tc, ExitStack() as ctx:
        consts = ctx.enter_context(tc.tile_pool(name="consts", bufs=1))
        coefp = ctx.enter_context(tc.tile_pool(name="coefs", bufs=1))
        srcp = ctx.enter_context(tc.tile_pool(name="srcs", bufs=1))
        stp = ctx.enter_context(tc.tile_pool(name="state", bufs=1))
        stgp = ctx.enter_context(tc.tile_pool(name="stage", bufs=2))
        psp = ctx.enter_context(tc.tile_pool(name="psum", bufs=2, space="PSUM"))

        shifts = consts.tile([128, 5, 128], F32, tag="shifts")
        nc.sync.dma_start(out=shifts[:], in_=par["shifts"][:])
        nhalfpi = consts.tile([128, 1], F32, tag="nhalfpi")
        nc.vector.memset(nhalfpi[:], -HALF_PI)

        SINP = (-0.16666666639369604, 0.0083333316715976, -0.00019840942043806986,
                2.752917460996653e-06, -2.3955613511594512e-08)
        COSP = (-0.49999999647064386, 0.041666645176626854, -0.0013888464831511677,
                2.4765157753536994e-05, -2.6136488530828197e-07)
        PI_HI = 3.1415927410125732
        PI_LO = -8.742278012618954e-08

        def cos_sin(c_out, s_out, th, tagp):
            """cos/sin of th in [0, 2pi] to ~1ulp f32, bias ~6e-10 rad.

            Quadrant reduction with double-f32 pi: both reduction branches are
            Sterbenz-exact in f32, the residual lo-part is a tiny constant
            selected by (sign, fold) masks and applied as a first-order
            rotation.  A plain fp32 reduction has ~2.6e-8 rad systematic bias
            which compounds coherently over ~1024 phase layers.
            """
            shape = list(th.shape)
            t_ = lambda nm: srcp.tile(shape, F32, tag=f"{tagp}{nm}", name=f"{tagp}{nm}")
            z, nz, a, mm, m2, p, acc, msk, w, sm = (
                t_(n) for n in ("z", "nz", "a", "mm", "m2", "p", "acc", "msk", "w", "sm"))
            v = nc.vector
            v.tensor_scalar(out=z[:], in0=th[:], scalar1=-PI_HI, scalar2=None, op0=ALU.add)
            v.tensor_scalar(out=nz[:], in0=z[:], scalar1=-1.0, scalar2=None, op0=ALU.mult)
            v.tensor_tensor(out=a[:], in0=nz[:], in1=z[:], op=ALU.max)          # |z|
            v.tensor_scalar(out=mm[:], in0=a[:], scalar1=-1.0, scalar2=PI_HI,
                            op0=ALU.mult, op1=ALU.add)                          # pi_hi-|z|
            v.tensor_scalar(out=msk[:], in0=mm[:], scalar1=1.0, scalar2=None,
                            op0=ALU.bypass)                                     # copy fold arm
            v.tensor_tensor(out=mm[:], in0=mm[:], in1=a[:], op=ALU.min)         # folded angle
            v.tensor_tensor(out=msk[:], in0=a[:], in1=msk[:], op=ALU.is_gt)     # a > pi_hi-a
            # mm_lo = PI_LO * (msk + sign(z)*(2*msk - 1))
            nc.scalar.sign(w[:], z[:])
            v.tensor_scalar(out=acc[:], in0=msk[:], scalar1=2.0, scalar2=-1.0,
                            op0=ALU.mult, op1=ALU.add)
            v.tensor_mul(w[:], w[:], acc[:])
            v.tensor_add(w[:], w[:], msk[:])
            v.tensor_scalar(out=w[:], in0=w[:], scalar1=PI_LO, scalar2=None, op0=ALU.mult)
            v.tensor_mul(m2[:], mm[:], mm[:])
            # sin(mm) = mm + mm^3 * P(m2)
            v.tensor_scalar(out=p[:], in0=m2[:], scalar1=SINP[4], scalar2=SINP[3],
                            op0=ALU.mult, op1=ALU.add)
            for cf in (SINP[2], SINP[1], SINP[0]):
                v.tensor_mul(p[:], p[:], m2[:])
                v.tensor_scalar(out=p[:], in0=p[:], scalar1=cf, scalar2=None, op0=ALU.add)
            v.tensor_mul(acc[:], m2[:], mm[:])
            v.tensor_mul(p[:], p[:], acc[:])
            v.tensor_add(sm[:], p[:], mm[:])                                    # sin(mm)
            # cos(mm) = 1 + m2 * Q(m2)
            v.tensor_scalar(out=p[:], in0=m2[:], scalar1=COSP[4], scalar2=COSP[3],
                            op0=ALU.mult, op1=ALU.add)
            for cf in (COSP[2], COSP[1], COSP[0]):
                v.tensor_mul(p[:], p[:], m2[:])
                v.tensor_scalar(out=p[:], in0=p[:], scalar1=cf, scalar2=None, op0=ALU.add)
            v.tensor_mul(p[:], p[:], m2[:])
            v.tensor_scalar(out=p[:], in0=p[:], scalar1=1.0, scalar2=None, op0=ALU.add)
            # first-order rotation by mm_lo, then quadrant signs
            v.tensor_mul(acc[:], w[:], p[:])        # mm_lo * cos
            v.tensor_add(acc[:], acc[:], sm[:])     # sin'
            v.tensor_mul(sm[:], w[:], sm[:])        # mm_lo * sin
            v.tensor_sub(p[:], p[:], sm[:])         # cos'
            nc.scalar.sign(s_out[:], nz[:])
            v.tensor_mul(s_out[:], s_out[:], acc[:])
            v.tensor_scalar(out=acc[:], in0=a[:], scalar1=-HALF_PI, scalar2=None,
                            op0=ALU.add)
            nc.scalar.sign(c_out[:], acc[:])
            v.tensor_mul(c_out[:], c_out[:], p[:])

        # ---- per-layer coefficient tiles, one partition per pair ----
        # groups: 0 = even pairs 0..127, 1 = even pairs 128..255,
        #         2 = odd pairs 0..127,  3 = odd pairs 128..255(pad)
        CO = {}
        for g, (thn, lon, ion) in enumerate(
            (("the0", "le0", "ie0"), ("the1", "le1", "ie1"),
             ("tho0", "lo0", "io0"), ("tho1", "lo1", "io1"))
        ):
            th = srcp.tile([128, L], F32, tag=f"th{g}")
            lo = srcp.tile([128, L], F32, tag=f"lo{g}")
            io = srcp.tile([128, L], F32, tag=f"io{g}")
            nc.sync.dma_start(out=th[:], in_=par[thn][:])
            nc.sync.dma_start(out=lo[:], in_=par[lon][:])
            nc.sync.dma_start(out=io[:], in_=par[ion][:])

            c_ = srcp.tile([128, L], F32, tag="c_")
            s_ = srcp.tile([128, L], F32, tag="s_")
            u_ = srcp.tile([128, L], F32, tag="u_")
            vp = srcp.tile([128, L], F32, tag="vp")
            vm = srcp.tile([128, L], F32, tag="vm")
            pp = srcp.tile([128, L], F32, tag="pp")
            pm = srcp.tile([128, L], F32, tag="pm")

            # clamp theta to [0, 2pi] (STE clamp forward value)
            nc.vector.tensor_scalar(out=th[:], in0=th[:], scalar1=0.0,
                                    scalar2=TWO_PI, op0=ALU.max, op1=ALU.min)
            cos_sin(c_, s_, th, "cs")
            # u = 1 - loss ; vp = 0.5 + imb ; vm = 0.5 - imb
            nc.vector.tensor_scalar(out=u_[:], in0=lo[:], scalar1=-1.0,
                                    scalar2=1.0, op0=ALU.mult, op1=ALU.add)
            nc.vector.tensor_scalar(out=vp[:], in0=io[:], scalar1=0.5,
                                    scalar2=None, op0=ALU.add)
            nc.vector.tensor_scalar(out=vm[:], in0=io[:], scalar1=-1.0,
                                    scalar2=0.5, op0=ALU.mult, op1=ALU.add)
            nc.vector.tensor_mul(pp[:], u_[:], vp[:])
            nc.vector.tensor_mul(pm[:], u_[:], vm[:])

            # one packed tile per group: coeff index  0:t 1:r 2:A 3:B 4:Bn 5:rn 6:E 7:En 8:Dn
            cg = coefp.tile([128, 9, L], F32, tag=f"cg{g}")
            t_ = cg[:, 0, :]; r_ = cg[:, 1, :]; A_ = cg[:, 2, :]
            B_ = cg[:, 3, :]; Bn = cg[:, 4, :]; rn = cg[:, 5, :]
            E_ = cg[:, 6, :]; En = cg[:, 7, :]; Dn = cg[:, 8, :]
            # ACT Sqrt is low precision (~1e-4); one Newton step fixes it:
            # y = 0.5*(y0 + x/y0), guarded against x=0 (odd pad pair has r=0).
            def sqrt_ref(dst, x, y0t, rec):
                nc.scalar.activation(y0t[:], x[:], ACTF.Sqrt)
                nc.vector.tensor_scalar(out=rec[:], in0=y0t[:], scalar1=1e-30,
                                        scalar2=None, op0=ALU.max)
                nc.vector.reciprocal(rec[:], rec[:])
                nc.vector.tensor_mul(rec[:], x[:], rec[:])
                nc.vector.tensor_add(rec[:], rec[:], y0t[:])
                nc.vector.tensor_scalar(out=dst, in0=rec[:], scalar1=0.5,
                                        scalar2=None, op0=ALU.mult)

            sq_y0 = srcp.tile([128, L], F32, tag="sqy")
            sq_rc = srcp.tile([128, L], F32, tag="sqr")
            sqrt_ref(t_, pp, sq_y0, sq_rc)
            sqrt_ref(r_, pm, sq_y0, sq_rc)
            nc.vector.tensor_mul(A_, t_, c_[:])
            nc.vector.tensor_mul(B_, t_, s_[:])
            nc.vector.tensor_mul(E_, r_, c_[:])
            # reuse pp as D = r*s scratch
            nc.vector.tensor_mul(pp[:], r_, s_[:])
            nc.vector.tensor_scalar(out=Dn, in0=pp[:], scalar1=-1.0,
                                    scalar2=None, op0=ALU.mult)
            nc.vector.tensor_scalar(out=Bn, in0=B_, scalar1=-1.0,
                                    scalar2=None, op0=ALU.mult)
            nc.vector.tensor_scalar(out=rn, in0=r_, scalar1=-1.0,
                                    scalar2=None, op0=ALU.mult)
            nc.vector.tensor_scalar(out=En, in0=E_, scalar1=-1.0,
                                    scalar2=None, op0=ALU.mult)
            CO[g] = cg

        # ---- state ----
        Ea = stp.tile([128, 2, 2, CPD], F32, tag="Ea")
        Oa = stp.tile([128, 2, 2, CPD], F32, tag="Oa")
        Eb = stp.tile([128, 2, 2, CPD], F32, tag="Eb")
        Ob = stp.tile([128, 2, 2, CPD], F32, tag="Ob")
        EsA = stp.tile([128, 2, 2, CPD], F32, tag="EsA")
        EsB = stp.tile([128, 2, 2, CPD], F32, tag="EsB")

        nc.vector.memset(Ea[:], 0.0)
        nc.vector.memset(Oa[:], 0.0)
        nc.sync.dma_start(out=Ea[:, :, 0, :], in_=par["init_e"][:])
        nc.sync.dma_start(out=Oa[:, :, 0, :], in_=par["init_o"][:])

        SU = shifts[:, 0, :]
        SB = shifts[:, 1, :]
        SD = shifts[:, 2, :]
        S00 = shifts[:, 3, :]
        SB127 = shifts[:, 4, :]

        CIDX = dict(t=0, r=1, A=2, B=3, Bn=4, rn=5, E=6, En=7, Dn=8)

        def mix(dst, srcT, srcB, stage, li):
            """One PC+MMI column: top rows srcT, bottom rows srcB -> dst.

            dst/srcT/srcB: dicts with APs xt,yt,xb,yb (each [128, CPD]).
            stage: [128, 9, 2U] staged coeffs; li: static in-stage index.
            """
            C = {k: stage[:, v, :] for k, v in CIDX.items()}

            def sl(T):
                return T[:, li : li + 1]

            v = nc.vector
            g_ = nc.gpsimd

            def lead_act(out, in_, coef):
                nc.scalar.activation(out, in_, ACTF.Copy, bias=0.0, scale=sl(coef))

            def stt(eng, out, in0, coef, in1):
                eng.scalar_tensor_tensor(out=out, in0=in0, scalar=sl(coef),
                                         in1=in1, op0=ALU.mult, op1=ALU.add)

            # Engine split: ACT takes the xt/yt leading multiplies, GpSimd the
            # merged bottom lead t*[xb|yb] (one [128,128] tensor_scalar; GpSimd
            # has no scalar_tensor_tensor opcode), DVE the 8 fused mul-adds.
            lead_act(dst["xt"], srcT["xt"], C["A"])
            lead_act(dst["yt"], srcT["xt"], C["B"])
            if "xyb" in srcB:
                lead_act(dst["xyb"], srcB["xyb"], C["t"])
            else:
                lead_act(dst["xb"], srcB["xb"], C["t"])
                lead_act(dst["yb"], srcB["yb"], C["t"])
            # X_top' = A xt + Bn yt + rn yb
            stt(v, dst["xt"], srcT["yt"], C["Bn"], dst["xt"])
            stt(v, dst["xt"], srcB["yb"], C["rn"], dst["xt"])
            # Y_top' = B xt + A yt + r xb
            stt(v, dst["yt"], srcT["yt"], C["A"], dst["yt"])
            stt(v, dst["yt"], srcB["xb"], C["r"], dst["yt"])
            # X_bot' = t xb + Dn xt + En yt
            stt(v, dst["xb"], srcT["xt"], C["Dn"], dst["xb"])
            stt(v, dst["xb"], srcT["yt"], C["En"], dst["xb"])
            # Y_bot' = t yb + E xt + Dn yt
            stt(v, dst["yb"], srcT["xt"], C["E"], dst["yb"])
            stt(v, dst["yb"], srcT["yt"], C["Dn"], dst["yb"])

        def even_layer(srcE, srcO, dstE, dstO, stages, li):
            for t in (0, 1):
                mix(
                    dict(xt=dstE[:, t, 0, :], yt=dstE[:, t, 1, :],
                         xb=dstO[:, t, 0, :], yb=dstO[:, t, 1, :],
                         xyb=dstO[:, t, :, :]),
                    dict(xt=srcE[:, t, 0, :], yt=srcE[:, t, 1, :]),
                    dict(xb=srcO[:, t, 0, :], yb=srcO[:, t, 1, :],
                         xyb=srcO[:, t, :, :]),
                    stages[t], li,
                )

        def odd_layer(srcO, botX, botY, dstO, dstEs, stages, li, botXY=None):
            # top = O[k], bottom = E[k+1] (pre-shifted into botX/botY APs)
            for t in (0, 1):
                mix(
                    dict(xt=dstO[:, t, 0, :], yt=dstO[:, t, 1, :],
                         xb=dstEs[:, t, 0, :], yb=dstEs[:, t, 1, :],
                         xyb=dstEs[:, t, :, :]),
                    dict(xt=srcO[:, t, 0, :], yt=srcO[:, t, 1, :]),
                    dict(xb=botX(t), yb=botY(t), xyb=botXY(t)),
                    stages[2 + t], li,
                )

        U = 4  # scan steps per loop iteration

        def body(j, u_steps=None):
            # j = base even/odd-layer index for this iteration (advances by 2U).
            # One dynamic-sliced copy per coeff group, then all static slices
            # (dynamic APs burn engine address registers: ~24/body max).
            if u_steps is None:
                u_steps = U
            stages = []
            for g in range(4):
                sg = stgp.tile([128, 9, 2 * U], F32, tag=f"stage{g}")
                nc.vector.tensor_copy(out=sg[:], in_=CO[g][:, :, bass.ds(j, 2 * U)])
                stages.append(sg)
            for u in range(u_steps):
                li0, li1 = 2 * u, 2 * u + 1
                even_layer(Ea, Oa, Eb, Ob, stages, li0)
                even_layer(Eb, Ob, Ea, Oa, stages, li1)

                # Esh[k] = E[k+1]  (linear over the two E tiles), built on PE
                psh = psp.tile([128, 2, 2, CPD], F32, tag="psh")
                nc.tensor.matmul(out=psh[:, 1, :, :], lhsT=SU, rhs=Ea[:, 1, :, :],
                                 start=True, stop=True)
                nc.tensor.matmul(out=psh[:, 0, :, :], lhsT=SU, rhs=Ea[:, 0, :, :],
                                 start=True, stop=False)
                nc.tensor.matmul(out=psh[:, 0, :, :], lhsT=SB, rhs=Ea[:, 1, :, :],
                                 start=False, stop=True)
                # PSUM -> SBUF so GpSimd chains can read it (and DVE avoids
                # the PSUM-source penalty)
                esh = stgp.tile([128, 2, 2, CPD], F32, tag="esh")
                nc.scalar.copy(out=esh[:], in_=psh[:])

                odd_layer(Oa, lambda t: esh[:, t, 0, :], lambda t: esh[:, t, 1, :],
                          Ob, EsB, stages, li0, botXY=lambda t: esh[:, t, :, :])
                odd_layer(Ob, lambda t: EsB[:, t, 0, :], lambda t: EsB[:, t, 1, :],
                          Oa, EsA, stages, li1, botXY=lambda t: EsB[:, t, :, :])

                # shift Es back: E'[k+1] = Es[k]; E'[0] = old E[0] (row 0 fixed)
                peb = psp.tile([128, 2, 2, CPD], F32, tag="peb")
                nc.tensor.matmul(out=peb[:, 0, :, :], lhsT=SD, rhs=EsA[:, 0, :, :],
                                 start=True, stop=False)
                nc.tensor.matmul(out=peb[:, 0, :, :], lhsT=S00, rhs=Ea[:, 0, :, :],
                                 start=False, stop=True)
                nc.tensor.matmul(out=peb[:, 1, :, :], lhsT=SD, rhs=EsA[:, 1, :, :],
                                 start=True, stop=False)
                nc.tensor.matmul(out=peb[:, 1, :, :], lhsT=SB127, rhs=EsA[:, 0, :, :],
                                 start=False, stop=True)
                nc.scalar.copy(out=Ea[:], in_=peb[:])

        if n_steps > 2:
            assert (2 * n_steps) % (2 * U) == 0
            with tc.For_i(0, n_steps * 2, 2 * U) as j:
                body(j)
        else:
            for k in range(n_steps):
                body(2 * k, u_steps=1)

        # ---- output phases and store ----
        po = consts.tile([128, 4], F32, tag="po")
        co = consts.tile([128, 4], F32, tag="co")
        so = consts.tile([128, 4], F32, tag="so")
        son = consts.tile([128, 4], F32, tag="son")
        nc.sync.dma_start(out=po[:], in_=par["pout"][:])
        nc.vector.tensor_scalar(out=po[:], in0=po[:], scalar1=0.0,
                                scalar2=TWO_PI, op0=ALU.max, op1=ALU.min)
        cos_sin(co, so, po, "csout")
        nc.vector.tensor_scalar(out=son[:], in0=so[:], scalar1=-1.0,
                                scalar2=None, op0=ALU.mult)

        fE = stp.tile([128, 2, 2, CPD], F32, tag="fE")
        fO = stp.tile([128, 2, 2, CPD], F32, tag="fO")
        for (S, D, c0) in ((Ea, fE, 0), (Oa, fO, 2)):
            for t in (0, 1):
                cs = co[:, c0 + t : c0 + t + 1]
                ss = so[:, c0 + t : c0 + t + 1]
                sn = son[:, c0 + t : c0 + t + 1]
                v = nc.vector
                v.tensor_scalar(out=D[:, t, 0, :], in0=S[:, t, 0, :],
                                scalar1=cs, scalar2=None, op0=ALU.mult)
                v.scalar_tensor_tensor(out=D[:, t, 0, :], in0=S[:, t, 1, :],
                                       scalar=sn, in1=D[:, t, 0, :],
                                       op0=ALU.mult, op1=ALU.add)
                v.tensor_scalar(out=D[:, t, 1, :], in0=S[:, t, 1, :],
                                scalar1=cs, scalar2=None, op0=ALU.mult)
                v.scalar_tensor_tensor(out=D[:, t, 1, :], in0=S[:, t, 0, :],
                                       scalar=ss, in1=D[:, t, 1, :],
                                       op0=ALU.mult, op1=ALU.add)
        nc.sync.dma_start(out=outv[:, 0, :, :, :], in_=fE[:])
        nc.sync.dma_start(out=outv[:, 1, :, :, :], in_=fO[:])

    nc.finalize()
    return nc


def _host_inputs(pc_even_phases, pc_odd_phases, pc_out_phases,
                 mmi_loss_even, mmi_imb_even, mmi_loss_odd, mmi_imb_odd,
                 n_steps=L // 2):
    f = np.float32
    thT = np.ascontiguousarray(pc_even_phases.T.astype(f))      # [256, 512]
    leT = np.ascontiguousarray(mmi_loss_even.T.astype(f))
    ieT = np.ascontiguousarray(mmi_imb_even.T.astype(f))

    tho = np.zeros((256, L), f)
    loo = np.zeros((256, L), f)
    ioo = np.zeros((256, L), f)
    tho[:255] = pc_odd_phases.T
    loo[:255] = mmi_loss_odd.T
    ioo[:255] = mmi_imb_odd.T
    ioo[255] = 0.5  # pad pair -> identity (t=1, r=0)

    shifts = np.zeros((128, 5, 128), f)
    for p in range(127):
        shifts[p + 1, 0, p] = 1.0     # SU: out[p] = in[p+1]
        shifts[p, 2, p + 1] = 1.0     # SD: out[p+1] = in[p]
    shifts[0, 1, 127] = 1.0           # SB: out[127] = in[0]
    shifts[0, 3, 0] = 1.0             # S00: out[0] = in[0]
    shifts[127, 4, 0] = 1.0           # SB127: out[0] = in[127]

    pout = np.zeros((128, 4), f)
    p = np.arange(128)
    pc = pc_out_phases.astype(f)
    pout[:, 0] = pc[2 * p]
    pout[:, 1] = pc[256 + 2 * p]
    pout[:, 2] = pc[2 * p + 1]
    pout[:, 3] = pc[257 + 2 * p]

    base = {
        "the0": thT[:128], "the1": thT[128:],
        "le0": leT[:128], "le1": leT[128:],
        "ie0": ieT[:128], "ie1": ieT[128:],
        "tho0": tho[:128], "tho1": tho[128:],
        "lo0": loo[:128], "lo1": loo[128:],
        "io0": ioo[:128], "io1": ioo[128:],
        "pout": pout, "shifts": shifts,
    }

    in_maps = []
    for d in range(NCORES):
        init_e = np.zeros((128, 2, CPD), f)
        init_o = np.zeros((128, 2, CPD), f)
        for j in range(CPD):
            row = CPD * d + j
            t, rr = divmod(row, 256)
            if rr % 2 == 0:
                init_e[rr // 2, t, j] = 1.0
            else:
                init_o[(rr - 1) // 2, t, j] = 1.0
        m = dict(base)
        m["init_e"] = init_e
        m["init_o"] = init_o
        in_maps.append(m)
    return in_maps


def _assemble(results):
    M = np.zeros((N, N), np.complex64)
    p = np.arange(128)
    for d in range(NCORES):
        o = results[d]["outv"]  # [128, 2(E/O), 2(X/Y), 2(t), CPD]
        cols = slice(CPD * d, CPD * (d + 1))
        for t in (0, 1):
            rE = 256 * t + 2 * p
            rO = 256 * t + 2 * p + 1
            M[rE, cols] = o[:, 0, t, 0, :] + 1j * o[:, 0, t, 1, :]
            M[rO, cols] = o[:, 1, t, 0, :] + 1j * o[:, 1, t, 1, :]
    return M


def _run(inputs, trace=False):
    if "nc" not in _CACHE:
        _CACHE["nc"] = _build_program()
    nc = _CACHE["nc"]
    inputs = {k: np.asarray(v) for k, v in inputs.items()}
    in_maps = _host_inputs(**inputs)
    try:
        res = run_bass_kernel_spmd(nc, in_maps, list(range(NCORES)), trace=trace)
    except Exception:
        # transient NRT_EXEC_UNIT_UNRECOVERABLE hiccups resolve on retry
        import time
        time.sleep(20)
        res = run_bass_kernel_spmd(nc, in_maps, list(range(NCORES)), trace=trace)
    return _assemble(res.results), res


def kernel(**inputs):
    return _run(inputs)[0]

